# revision 3
# baseline (speedup 1.0000x reference)
"""Dice-loss kernel for Trainium2 (Bass/Tile), 8-core data-parallel SPMD.

Strategy
--------
reference: pred = argmax_c(logits); for c in 1..4:
    inter_c = #{v : pred[v]==c and tgt[v]==c},  tsum_c = #{v : tgt[v]==c}
    dice_c = (2*inter_c + eps) / (inter_c + tsum_c + eps); loss = 1 - mean(dice)

The voxel axis (B*D*H*W = 7,077,888) is sharded 8 ways; each core gets
[5, 128, 6912] fp16 logits and [128, 6912] fp16 labels.  On device (all DVE):
  m   = max of the 5 class planes            (4 tensor_tensor max)
  e_c = (l_c >= m)                           (4 tensor_tensor is_ge)
  t_c = (tgt == c)  + free-axis sum -> tsum  (4 tensor_scalar, 4x mode)
  i_c = t_c * e_c   + free-axis sum -> inter (4 scalar_tensor_tensor)
Per-partition partial sums [128, 8] go back to the host, which reduces
across partitions/cores and evaluates the scalar dice in float32.

fp16 note: logits are converted to fp16 on the host.  argmax ties after
fp16 rounding affect ~0.03% of voxels, giving ~1e-4 relative error on the
loss (the check tolerance is far looser).  Counts stay exact integers in
fp32 accumulators.
"""

import sys
from contextlib import ExitStack

import numpy as np

for _p in ("/opt/trn_rl_repo", "/opt/pypackages"):
    if _p not in sys.path:
        sys.path.append(_p)

import concourse.bacc as bacc
import concourse.tile as tile
from concourse import mybir
from concourse.bass_utils import run_bass_kernel_spmd

# Problem shape (hardcoded per contract: kernel.py must be self-contained).
B, C, D, H, W = 2, 5, 96, 192, 192
N_CORES = 8
P = 128                      # SBUF partitions
NVOX = B * D * H * W         # 7,077,888 voxels
SHARD = NVOX // N_CORES      # 884,736 voxels per core
FTOT = SHARD // P            # 6,912 free elems per partition
FD = 2304                    # free elems per tile
NT = FTOT // FD              # 3 tiles
NCLS = C - 1                 # foreground classes 1..4
NQ = 2 * NCLS                # 4 inter + 4 tsum accumulators
EPS = 1e-8


def emit_dice_kernel(tc, logits_ap, tgt_ap, partials_ap, n_cls, p, ftot, fd):
    """Emit the per-core dice partial-sums program into TileContext `tc`.

    logits_ap:   DRAM [C, p, ftot] fp16
    tgt_ap:      DRAM [p, ftot]    fp16 (labels 0..C-1, exact)
    partials_ap: DRAM [p, 2*(C-1)] f32; cols 0..C-2 inter_c, C-1..2C-3 tsum_c
    """
    nc = tc.nc
    n_cls_total = n_cls + 1  # C
    nt = ftot // fd
    assert nt * fd == ftot
    nq = 2 * n_cls
    fp16 = mybir.dt.float16
    f32 = mybir.dt.float32
    Alu = mybir.AluOpType

    with ExitStack() as ctx:
        pool_in = ctx.enter_context(tc.tile_pool(name="in", bufs=2))
        pool_tmp = ctx.enter_context(tc.tile_pool(name="tmp", bufs=1))
        pool_acc = ctx.enter_context(tc.tile_pool(name="acc", bufs=1))

        # Per-tile accumulator columns: [p, quantity, tile].
        acc = pool_acc.tile([p, nq * nt], f32, tag="acc")
        accf = pool_acc.tile([p, nq], f32, tag="accf")

        for i in range(nt):
            sl = slice(i * fd, (i + 1) * fd)
            lg = []
            for c in range(n_cls_total):
                t = pool_in.tile([p, fd], fp16, tag=f"lg{c}")
                nc.sync.dma_start(out=t, in_=logits_ap[c, :, sl])
                lg.append(t)
            tg = pool_in.tile([p, fd], fp16, tag="tg")
            nc.sync.dma_start(out=tg, in_=tgt_ap[:, sl])

            # m = max over the 5 class planes (4 TT max ops)
            ma = pool_tmp.tile([p, fd], fp16, tag="ma")
            mb = pool_tmp.tile([p, fd], fp16, tag="mb")
            m = pool_tmp.tile([p, fd], fp16, tag="m")
            nc.vector.tensor_tensor(ma, lg[0], lg[1], Alu.max)
            nc.vector.tensor_tensor(mb, lg[2], lg[3], Alu.max)
            nc.vector.tensor_tensor(ma, ma, mb, Alu.max)
            nc.vector.tensor_tensor(m, ma, lg[4], Alu.max)

            e = pool_tmp.tile([p, fd], fp16, tag="e")
            t_c = pool_tmp.tile([p, fd], fp16, tag="t_c")
            dump = pool_tmp.tile([p, fd], fp16, tag="dump")
            for c in range(1, n_cls_total):
                q_inter = c - 1
                q_tsum = n_cls + (c - 1)
                # e_c = (l_c >= m): 1.0 iff class c attains the max
                nc.vector.tensor_tensor(e, lg[c], m, Alu.is_ge)
                # t_c = (tgt == c), accum(op1=add) -> per-partition tsum partial
                nc.vector.tensor_scalar(
                    t_c,
                    tg,
                    float(c),
                    None,
                    Alu.is_equal,
                    Alu.add,
                    accum_out=acc[:, q_tsum * nt + i : q_tsum * nt + i + 1],
                )
                # (t_c * 1.0) * e_c, accum -> per-partition inter partial
                nc.vector.scalar_tensor_tensor(
                    dump,
                    t_c,
                    1.0,
                    e,
                    Alu.mult,
                    Alu.mult,
                    accum_out=acc[:, q_inter * nt + i : q_inter * nt + i + 1],
                )

        # Sum the per-tile columns: [p, nq, nt] -> [p, nq]
        acc3 = acc.rearrange("p (q t) -> p q t", q=nq)
        nc.vector.tensor_reduce(accf, acc3, mybir.AxisListType.X, Alu.add)
        nc.sync.dma_start(out=partials_ap, in_=accf)


_PROGRAM_CACHE = {}


def build_program():
    key = (C, P, FTOT, FD)
    if key in _PROGRAM_CACHE:
        return _PROGRAM_CACHE[key]
    nc = bacc.Bacc("TRN2", debug=False, target_bir_lowering=False)
    logits = nc.dram_tensor(
        "logits", [C, P, FTOT], mybir.dt.float16, kind="ExternalInput"
    )
    tgt = nc.dram_tensor("tgt", [P, FTOT], mybir.dt.float16, kind="ExternalInput")
    partials = nc.dram_tensor(
        "partials", [P, NQ], mybir.dt.float32, kind="ExternalOutput"
    )
    with tile.TileContext(nc) as tc:
        emit_dice_kernel(tc, logits.ap(), tgt.ap(), partials.ap(), NCLS, P, FTOT, FD)
    nc.compile()
    _PROGRAM_CACHE[key] = nc
    return nc


def make_in_maps(input2, target1):
    lg16 = np.asarray(input2, dtype=np.float32).astype(np.float16)
    tg16 = np.asarray(target1).astype(np.float16)
    lgf = lg16.reshape(B, C, NVOX // B)
    tgf = tg16.reshape(B, NVOX // B)
    shards_per_b = N_CORES // B
    s = (NVOX // B) // shards_per_b
    in_maps = []
    for core in range(N_CORES):
        b, q = divmod(core, shards_per_b)
        sl = slice(q * s, (q + 1) * s)
        in_maps.append(
            {
                "logits": np.ascontiguousarray(lgf[b, :, sl]).reshape(C, P, FTOT),
                "tgt": np.ascontiguousarray(tgf[b, sl]).reshape(P, FTOT),
            }
        )
    return in_maps


def _finish(partials_list):
    """Host-side reduction: [P, 8] f32 per core -> scalar loss (float32)."""
    total = np.zeros(NQ, dtype=np.float64)
    for parts in partials_list:
        total += parts.astype(np.float64).sum(axis=0)
    inter = total[:NCLS].astype(np.float32)
    tsum = total[NCLS:].astype(np.float32)
    eps = np.float32(EPS)
    dice = (np.float32(2.0) * inter + eps) / (inter + tsum + eps)
    loss = np.float32(1.0) - np.mean(dice, dtype=np.float32)
    return np.array([loss], dtype=np.float32)


# test.py can set e.g. RUN_KWARGS.update(trace=True) to profile; the grader
# path leaves this empty.
RUN_KWARGS = {}
LAST_RESULT = None


def kernel(input2, target1):
    global LAST_RESULT
    nc = build_program()
    in_maps = make_in_maps(input2, target1)
    res = run_bass_kernel_spmd(nc, in_maps, core_ids=list(range(N_CORES)), **RUN_KWARGS)
    LAST_RESULT = res
    return _finish([r["partials"] for r in res.results])


# revision 15
# speedup vs baseline: 1.4198x; 1.4198x over previous
"""Dice-loss kernel for Trainium2 (Bass/Tile), 8-core data-parallel SPMD.

Strategy
--------
reference: pred = argmax_c(logits); for c in 1..4:
    inter_c = #{v : pred[v]==c and tgt[v]==c},  tsum_c = #{v : tgt[v]==c}
    dice_c = (2*inter_c + eps) / (inter_c + tsum_c + eps); loss = 1 - mean(dice)

The voxel axis (B*D*H*W = 7,077,888) is sharded 8 ways; each core gets
[5, 128, 6912] fp16 logits and [128, 6912] fp16 labels.  On device (all DVE):
  m   = max of the 5 class planes            (4 tensor_tensor max)
  e_c = (l_c >= m)                           (4 tensor_tensor is_ge)
  t_c = (tgt == c)  + free-axis sum -> tsum  (4 tensor_scalar, 4x mode)
  i_c = t_c * e_c   + free-axis sum -> inter (4 scalar_tensor_tensor)
Per-partition partial sums [128, 8] go back to the host, which reduces
across partitions/cores and evaluates the scalar dice in float32.

fp16 note: logits are converted to fp16 on the host.  argmax ties after
fp16 rounding affect ~0.03% of voxels, giving ~1e-4 relative error on the
loss (the check tolerance is far looser).  Counts stay exact integers in
fp32 accumulators.
"""

import sys
from contextlib import ExitStack

import numpy as np

for _p in ("/opt/trn_rl_repo", "/opt/pypackages"):
    if _p not in sys.path:
        sys.path.append(_p)

import concourse.bacc as bacc
import concourse.tile as tile
from concourse import mybir
from concourse.bass_utils import run_bass_kernel_spmd

# Problem shape (hardcoded per contract: kernel.py must be self-contained).
B, C, D, H, W = 2, 5, 96, 192, 192
N_CORES = 8
P = 128                      # SBUF partitions
NVOX = B * D * H * W         # 7,077,888 voxels
SHARD = NVOX // N_CORES      # 884,736 voxels per core
FTOT = SHARD // P            # 6,912 free elems per partition
FD = 2304                    # free elems per tile
NT = FTOT // FD              # 3 tiles
NCLS = C - 1                 # foreground classes 1..4
NQ = 2 * NCLS                # 4 inter + 4 tsum accumulators
EPS = 1e-8


def emit_dice_kernel(tc, logits_ap, tgt_ap, partials_ap, psums_ap, n_cls, p, ftot, fd):
    """Emit the per-core dice partial-sums program into TileContext `tc`.

    logits_ap:   DRAM [C, p, ftot] fp16
    tgt_ap:      DRAM [p, ftot]    fp16 (labels 0..C-1, exact)
    partials_ap: DRAM [p, 4*nt]    f32 -- ACT accum columns, layout
                 q*nt + i with q in {inter_1, inter_2, tsum_1, tsum_2}
    psums_ap:    DRAM [4, 512]     f32 -- PE PSUM rows
                 {inter_3, inter_4, tsum_3, tsum_4}

    DVE: max tree, is_ge, is_eq, mult.  ACT: 4 plane-sums/tile (classes 1,2).
    PE: 4 plane-sums/tile via ones-matmul PSUM accumulation (classes 3,4).
    """
    nc = tc.nc
    n_cls_total = n_cls + 1  # C
    nt = ftot // fd
    assert nt * fd == ftot
    fp16 = mybir.dt.float16
    f32 = mybir.dt.float32
    Alu = mybir.AluOpType
    Act = mybir.ActivationFunctionType

    # 512-wide matmul chunks (PSUM bank limit)
    ps_w = min(512, fd)
    chunks = []
    off = 0
    while off < fd:
        w = min(512, fd - off)
        chunks.append((off, w))
        off += w

    with ExitStack() as ctx:
        pool_in = ctx.enter_context(tc.tile_pool(name="in", bufs=2))
        pool_t1 = ctx.enter_context(tc.tile_pool(name="t1", bufs=1))
        pool_t2 = ctx.enter_context(tc.tile_pool(name="t2", bufs=2))
        pool_acc = ctx.enter_context(tc.tile_pool(name="acc", bufs=1))
        pool_ps = ctx.enter_context(tc.tile_pool(name="ps", bufs=1, space="PSUM"))

        acc = pool_acc.tile([p, 4 * nt], f32, tag="acc")
        ones = pool_acc.tile([p, 1], fp16, tag="ones")
        nc.vector.memset(ones, 1.0)
        # 4 PSUM banks: inter_3, inter_4, tsum_3, tsum_4
        ps = [
            pool_ps.tile([1, ps_w], f32, tag=f"ps{q}", name=f"ps{q}") for q in range(4)
        ]

        for i in range(nt):
            sl = slice(i * fd, (i + 1) * fd)
            lg = []
            for c in range(n_cls_total):
                t = pool_in.tile([p, fd], fp16, tag=f"lg{c}")
                nc.sync.dma_start(out=t, in_=logits_ap[c, :, sl])
                lg.append(t)
            tg = pool_in.tile([p, fd], fp16, tag="tg")
            nc.sync.dma_start(out=tg, in_=tgt_ap[:, sl])

            # m = max over the 5 class planes (4 TT max ops)
            ma = pool_t1.tile([p, fd], fp16, tag="ma")
            mb = pool_t1.tile([p, fd], fp16, tag="mb")
            m = pool_t1.tile([p, fd], fp16, tag="m")
            nc.vector.tensor_tensor(ma, lg[0], lg[1], Alu.max)
            nc.vector.tensor_tensor(mb, lg[2], lg[3], Alu.max)
            nc.vector.tensor_tensor(ma, ma, mb, Alu.max)
            nc.vector.tensor_tensor(m, ma, lg[4], Alu.max)

            dump = pool_t1.tile([p, fd], fp16, tag="dump")
            for c in range(1, n_cls_total):
                cls_i = c - 1  # 0..3
                on_pe = cls_i >= 2
                e = pool_t1.tile([p, fd], fp16, tag=f"e{c}")
                nc.vector.tensor_tensor(e, lg[c], m, Alu.is_ge)
                t_c = pool_t2.tile([p, fd], fp16, tag=f"t{c}")
                nc.vector.tensor_scalar(t_c, tg, float(c), None, Alu.is_equal)
                a_c = pool_t2.tile([p, fd], fp16, tag=f"a{c}")
                nc.vector.tensor_tensor(a_c, t_c, e, Alu.mult)

                if on_pe:
                    q_i, q_t = cls_i - 2, cls_i  # psum rows: 0,1 inter; 2,3 tsum
                    first = i == 0
                    last = i == nt - 1
                    for k, (o, w) in enumerate(chunks):
                        nc.tensor.matmul(
                            ps[q_i][:, 0:w],
                            ones,
                            a_c[:, o : o + w],
                            start=(first and k == 0),
                            stop=(last and k == len(chunks) - 1),
                        )
                    for k, (o, w) in enumerate(chunks):
                        nc.tensor.matmul(
                            ps[q_t][:, 0:w],
                            ones,
                            t_c[:, o : o + w],
                            start=(first and k == 0),
                            stop=(last and k == len(chunks) - 1),
                        )
                else:
                    # ACT copy-accum: inter cols q=cls_i, tsum cols q=2+cls_i
                    nc.scalar.activation(
                        dump,
                        a_c,
                        Act.Copy,
                        accum_out=acc[:, cls_i * nt + i : cls_i * nt + i + 1],
                    )
                    nc.scalar.activation(
                        dump,
                        t_c,
                        Act.Copy,
                        accum_out=acc[
                            :, (2 + cls_i) * nt + i : (2 + cls_i) * nt + i + 1
                        ],
                    )

        nc.sync.dma_start(out=partials_ap, in_=acc)
        # PSUM is not DMA-able: stage through SBUF via ACT copies.
        psout = pool_acc.tile([1, 4 * ps_w], f32, tag="psout")
        for q in range(4):
            nc.scalar.activation(
                psout[:, q * ps_w : (q + 1) * ps_w], ps[q], Act.Copy
            )
        nc.sync.dma_start(out=psums_ap, in_=psout)


_PROGRAM_CACHE = {}


def build_program():
    key = (C, P, FTOT, FD)
    if key in _PROGRAM_CACHE:
        return _PROGRAM_CACHE[key]
    nc = bacc.Bacc("TRN2", debug=False, target_bir_lowering=False)
    logits = nc.dram_tensor(
        "logits", [C, P, FTOT], mybir.dt.float16, kind="ExternalInput"
    )
    tgt = nc.dram_tensor("tgt", [P, FTOT], mybir.dt.float16, kind="ExternalInput")
    partials = nc.dram_tensor(
        "partials", [P, 4 * NT], mybir.dt.float32, kind="ExternalOutput"
    )
    psums = nc.dram_tensor(
        "psums", [1, 4 * min(512, FD)], mybir.dt.float32, kind="ExternalOutput"
    )
    with tile.TileContext(nc) as tc:
        emit_dice_kernel(
            tc, logits.ap(), tgt.ap(), partials.ap(), psums.ap(), NCLS, P, FTOT, FD
        )
    nc.compile()
    _PROGRAM_CACHE[key] = nc
    return nc


def make_in_maps(input2, target1):
    lg16 = np.asarray(input2, dtype=np.float32).astype(np.float16)
    tg16 = np.asarray(target1).astype(np.float16)
    lgf = lg16.reshape(B, C, NVOX // B)
    tgf = tg16.reshape(B, NVOX // B)
    shards_per_b = N_CORES // B
    s = (NVOX // B) // shards_per_b
    in_maps = []
    for core in range(N_CORES):
        b, q = divmod(core, shards_per_b)
        sl = slice(q * s, (q + 1) * s)
        in_maps.append(
            {
                "logits": np.ascontiguousarray(lgf[b, :, sl]).reshape(C, P, FTOT),
                "tgt": np.ascontiguousarray(tgf[b, sl]).reshape(P, FTOT),
            }
        )
    return in_maps


def _finish(results):
    """Host-side reduction of per-core partials -> scalar loss (float32).

    partials [P, 4*NT] cols q*NT+i, q in {inter_1, inter_2, tsum_1, tsum_2};
    psums [4, 512] rows {inter_3, inter_4, tsum_3, tsum_4}.
    """
    total = np.zeros(NQ, dtype=np.float64)  # inter_1..4, tsum_1..4
    for r in results:
        pa = r["partials"].astype(np.float64).reshape(P, 4, NT).sum(axis=(0, 2))
        pe = r["psums"].astype(np.float64).reshape(4, -1).sum(axis=1)
        total[0] += pa[0]  # inter_1
        total[1] += pa[1]  # inter_2
        total[2] += pe[0]  # inter_3
        total[3] += pe[1]  # inter_4
        total[4] += pa[2]  # tsum_1
        total[5] += pa[3]  # tsum_2
        total[6] += pe[2]  # tsum_3
        total[7] += pe[3]  # tsum_4
    inter = total[:NCLS].astype(np.float32)
    tsum = total[NCLS:].astype(np.float32)
    eps = np.float32(EPS)
    dice = (np.float32(2.0) * inter + eps) / (inter + tsum + eps)
    loss = np.float32(1.0) - np.mean(dice, dtype=np.float32)
    return np.array([loss], dtype=np.float32)


# test.py can set e.g. RUN_KWARGS.update(trace=True) to profile; the grader
# path leaves this empty.
RUN_KWARGS = {}
LAST_RESULT = None


def kernel(input2, target1):
    global LAST_RESULT
    nc = build_program()
    in_maps = make_in_maps(input2, target1)
    res = run_bass_kernel_spmd(nc, in_maps, core_ids=list(range(N_CORES)), **RUN_KWARGS)
    LAST_RESULT = res
    return _finish(res.results)


# revision 18
# speedup vs baseline: 1.4760x; 1.0395x over previous
"""Dice-loss kernel for Trainium2 (Bass/Tile), 8-core data-parallel SPMD.

Strategy
--------
reference: pred = argmax_c(logits); for c in 1..4:
    inter_c = #{v : pred[v]==c and tgt[v]==c},  tsum_c = #{v : tgt[v]==c}
    dice_c = (2*inter_c + eps) / (inter_c + tsum_c + eps); loss = 1 - mean(dice)

The voxel axis (B*D*H*W = 7,077,888) is sharded 8 ways; each core gets
[5, 128, 6912] fp16 logits and [128, 6912] fp16 labels.  On device (all DVE):
  m   = max of the 5 class planes            (4 tensor_tensor max)
  e_c = (l_c >= m)                           (4 tensor_tensor is_ge)
  t_c = (tgt == c)  + free-axis sum -> tsum  (4 tensor_scalar, 4x mode)
  i_c = t_c * e_c   + free-axis sum -> inter (4 scalar_tensor_tensor)
Per-partition partial sums [128, 8] go back to the host, which reduces
across partitions/cores and evaluates the scalar dice in float32.

fp16 note: logits are converted to fp16 on the host.  argmax ties after
fp16 rounding affect ~0.03% of voxels, giving ~1e-4 relative error on the
loss (the check tolerance is far looser).  Counts stay exact integers in
fp32 accumulators.
"""

import sys
from contextlib import ExitStack

import numpy as np

for _p in ("/opt/trn_rl_repo", "/opt/pypackages"):
    if _p not in sys.path:
        sys.path.append(_p)

import concourse.bacc as bacc
import concourse.tile as tile
from concourse import mybir
from concourse.bass_utils import run_bass_kernel_spmd

# Problem shape (hardcoded per contract: kernel.py must be self-contained).
B, C, D, H, W = 2, 5, 96, 192, 192
N_CORES = 8
P = 128                      # SBUF partitions
NVOX = B * D * H * W         # 7,077,888 voxels
SHARD = NVOX // N_CORES      # 884,736 voxels per core
FTOT = SHARD // P            # 6,912 free elems per partition
# Uneven tiling: small first tile starts compute sooner, small last tile
# shortens the PE/ACT tail.
TILES = [1152, 2304, 2304, 1152]
NT = len(TILES)
PS_W = 512
NCLS = C - 1                 # foreground classes 1..4
NQ = 2 * NCLS                # 4 inter + 4 tsum accumulators
EPS = 1e-8
assert sum(TILES) == FTOT


def emit_dice_kernel(tc, logits_ap, tgt_ap, partials_ap, psums_ap, n_cls, p, tiles):
    """Emit the per-core dice partial-sums program into TileContext `tc`.

    logits_ap:   DRAM [C, p, ftot] fp16
    tgt_ap:      DRAM [p, ftot]    fp16 (labels 0..C-1, exact)
    partials_ap: DRAM [p, 4*nt]    f32 -- ACT accum columns, layout
                 q*nt + i with q in {inter_1, inter_2, tsum_1, tsum_2}
    psums_ap:    DRAM [1, 4*ps_w]  f32 -- PE PSUM rows
                 {inter_3, inter_4, tsum_3, tsum_4}
    tiles:       list of free-dim tile sizes (uneven allowed; small first
                 tile starts compute sooner, small last tile shortens the
                 PE/ACT tail after DVE finishes)

    DVE: max tree, is_ge, is_eq, mult.  ACT: 4 plane-sums/tile (classes 1,2).
    PE: 4 plane-sums/tile via ones-matmul PSUM accumulation (classes 3,4).
    """
    nc = tc.nc
    n_cls_total = n_cls + 1  # C
    nt = len(tiles)
    fdmax = max(tiles)
    fp16 = mybir.dt.float16
    f32 = mybir.dt.float32
    Alu = mybir.AluOpType
    Act = mybir.ActivationFunctionType

    ps_w = min(512, fdmax)

    def chunk_list(fd):
        out, off = [], 0
        while off < fd:
            w = min(512, fd - off)
            out.append((off, w))
            off += w
        return out

    with ExitStack() as ctx:
        pool_in = ctx.enter_context(tc.tile_pool(name="in", bufs=2))
        pool_t1 = ctx.enter_context(tc.tile_pool(name="t1", bufs=1))
        pool_t2 = ctx.enter_context(tc.tile_pool(name="t2", bufs=2))
        pool_acc = ctx.enter_context(tc.tile_pool(name="acc", bufs=1))
        pool_ps = ctx.enter_context(tc.tile_pool(name="ps", bufs=1, space="PSUM"))

        acc = pool_acc.tile([p, 4 * nt], f32, tag="acc")
        ones = pool_acc.tile([p, 1], fp16, tag="ones")
        nc.vector.memset(ones, 1.0)
        # 4 PSUM banks: inter_3, inter_4, tsum_3, tsum_4
        ps = [
            pool_ps.tile([1, ps_w], f32, tag=f"ps{q}", name=f"ps{q}") for q in range(4)
        ]

        base = 0
        for i, fd in enumerate(tiles):
            sl = slice(base, base + fd)
            base += fd
            chunks = chunk_list(fd)
            # target first: the t_c tensor_scalar ops need it early
            tg = pool_in.tile([p, fdmax], fp16, tag="tg")
            nc.sync.dma_start(out=tg[:, 0:fd], in_=tgt_ap[:, sl])
            lg = []
            for c in range(n_cls_total):
                t = pool_in.tile([p, fdmax], fp16, tag=f"lg{c}")
                nc.sync.dma_start(out=t[:, 0:fd], in_=logits_ap[c, :, sl])
                lg.append(t)

            # m = max over the 5 class planes (4 TT max ops)
            ma = pool_t1.tile([p, fdmax], fp16, tag="ma")
            mb = pool_t1.tile([p, fdmax], fp16, tag="mb")
            m = pool_t1.tile([p, fdmax], fp16, tag="m")
            nc.vector.tensor_tensor(ma[:, 0:fd], lg[0][:, 0:fd], lg[1][:, 0:fd], Alu.max)
            nc.vector.tensor_tensor(mb[:, 0:fd], lg[2][:, 0:fd], lg[3][:, 0:fd], Alu.max)
            nc.vector.tensor_tensor(ma[:, 0:fd], ma[:, 0:fd], mb[:, 0:fd], Alu.max)
            nc.vector.tensor_tensor(m[:, 0:fd], ma[:, 0:fd], lg[4][:, 0:fd], Alu.max)

            dump = pool_t1.tile([p, fdmax], fp16, tag="dump")
            for c in range(1, n_cls_total):
                cls_i = c - 1  # 0..3
                on_pe = cls_i >= 2
                e = pool_t1.tile([p, fdmax], fp16, tag=f"e{c}")
                nc.vector.tensor_tensor(e[:, 0:fd], lg[c][:, 0:fd], m[:, 0:fd], Alu.is_ge)
                t_c = pool_t2.tile([p, fdmax], fp16, tag=f"t{c}")
                nc.vector.tensor_scalar(t_c[:, 0:fd], tg[:, 0:fd], float(c), None, Alu.is_equal)
                a_c = pool_t2.tile([p, fdmax], fp16, tag=f"a{c}")
                nc.vector.tensor_tensor(a_c[:, 0:fd], t_c[:, 0:fd], e[:, 0:fd], Alu.mult)

                if on_pe:
                    q_i, q_t = cls_i - 2, cls_i  # psum rows: 0,1 inter; 2,3 tsum
                    first = i == 0
                    last = i == nt - 1
                    for k, (o, w) in enumerate(chunks):
                        nc.tensor.matmul(
                            ps[q_i][:, 0:w],
                            ones,
                            a_c[:, o : o + w],
                            start=(first and k == 0),
                            stop=(last and k == len(chunks) - 1),
                        )
                    for k, (o, w) in enumerate(chunks):
                        nc.tensor.matmul(
                            ps[q_t][:, 0:w],
                            ones,
                            t_c[:, o : o + w],
                            start=(first and k == 0),
                            stop=(last and k == len(chunks) - 1),
                        )
                else:
                    # ACT copy-accum: inter cols q=cls_i, tsum cols q=2+cls_i
                    nc.scalar.activation(
                        dump[:, 0:fd],
                        a_c[:, 0:fd],
                        Act.Copy,
                        accum_out=acc[:, cls_i * nt + i : cls_i * nt + i + 1],
                    )
                    nc.scalar.activation(
                        dump[:, 0:fd],
                        t_c[:, 0:fd],
                        Act.Copy,
                        accum_out=acc[
                            :, (2 + cls_i) * nt + i : (2 + cls_i) * nt + i + 1
                        ],
                    )

        nc.sync.dma_start(out=partials_ap, in_=acc)
        # PSUM is not DMA-able: stage through SBUF (split DVE/ACT, both idle
        # at the tail).
        psout = pool_acc.tile([1, 4 * ps_w], f32, tag="psout")
        for q in range(4):
            dst = psout[:, q * ps_w : (q + 1) * ps_w]
            if q < 2:
                nc.vector.tensor_copy(dst, ps[q])
            else:
                nc.scalar.activation(dst, ps[q], Act.Copy)
        nc.sync.dma_start(out=psums_ap, in_=psout)


_PROGRAM_CACHE = {}


def build_program():
    key = (C, P, FTOT, tuple(TILES))
    if key in _PROGRAM_CACHE:
        return _PROGRAM_CACHE[key]
    nc = bacc.Bacc("TRN2", debug=False, target_bir_lowering=False)
    logits = nc.dram_tensor(
        "logits", [C, P, FTOT], mybir.dt.float16, kind="ExternalInput"
    )
    tgt = nc.dram_tensor("tgt", [P, FTOT], mybir.dt.float16, kind="ExternalInput")
    partials = nc.dram_tensor(
        "partials", [P, 4 * NT], mybir.dt.float32, kind="ExternalOutput"
    )
    psums = nc.dram_tensor(
        "psums", [1, 4 * PS_W], mybir.dt.float32, kind="ExternalOutput"
    )
    with tile.TileContext(nc) as tc:
        emit_dice_kernel(
            tc, logits.ap(), tgt.ap(), partials.ap(), psums.ap(), NCLS, P, TILES
        )
    nc.compile()
    _PROGRAM_CACHE[key] = nc
    return nc


def make_in_maps(input2, target1):
    lg16 = np.asarray(input2, dtype=np.float32).astype(np.float16)
    tg16 = np.asarray(target1).astype(np.float16)
    lgf = lg16.reshape(B, C, NVOX // B)
    tgf = tg16.reshape(B, NVOX // B)
    shards_per_b = N_CORES // B
    s = (NVOX // B) // shards_per_b
    in_maps = []
    for core in range(N_CORES):
        b, q = divmod(core, shards_per_b)
        sl = slice(q * s, (q + 1) * s)
        in_maps.append(
            {
                "logits": np.ascontiguousarray(lgf[b, :, sl]).reshape(C, P, FTOT),
                "tgt": np.ascontiguousarray(tgf[b, sl]).reshape(P, FTOT),
            }
        )
    return in_maps


def _finish(results):
    """Host-side reduction of per-core partials -> scalar loss (float32).

    partials [P, 4*NT] cols q*NT+i, q in {inter_1, inter_2, tsum_1, tsum_2};
    psums [4, 512] rows {inter_3, inter_4, tsum_3, tsum_4}.
    """
    total = np.zeros(NQ, dtype=np.float64)  # inter_1..4, tsum_1..4
    for r in results:
        pa = r["partials"].astype(np.float64).reshape(P, 4, NT).sum(axis=(0, 2))
        pe = r["psums"].astype(np.float64).reshape(4, -1).sum(axis=1)
        total[0] += pa[0]  # inter_1
        total[1] += pa[1]  # inter_2
        total[2] += pe[0]  # inter_3
        total[3] += pe[1]  # inter_4
        total[4] += pa[2]  # tsum_1
        total[5] += pa[3]  # tsum_2
        total[6] += pe[2]  # tsum_3
        total[7] += pe[3]  # tsum_4
    inter = total[:NCLS].astype(np.float32)
    tsum = total[NCLS:].astype(np.float32)
    eps = np.float32(EPS)
    dice = (np.float32(2.0) * inter + eps) / (inter + tsum + eps)
    loss = np.float32(1.0) - np.mean(dice, dtype=np.float32)
    return np.array([loss], dtype=np.float32)


# test.py can set e.g. RUN_KWARGS.update(trace=True) to profile; the grader
# path leaves this empty.
RUN_KWARGS = {}
LAST_RESULT = None


def kernel(input2, target1):
    global LAST_RESULT
    nc = build_program()
    in_maps = make_in_maps(input2, target1)
    res = run_bass_kernel_spmd(nc, in_maps, core_ids=list(range(N_CORES)), **RUN_KWARGS)
    LAST_RESULT = res
    return _finish(res.results)
